# revision 46
# baseline (speedup 1.0000x reference)
"""Distributed multi-head attention kernel for one TRN2 chip (8 NeuronCores).

Problem: x[4, 2048, 1024] -> qkv Linear(1024, 3072, bias=False) -> 16-head
softmax attention -> proj Linear(1024, 1024) + bias.

Sharding: tensor-parallel over heads. Core c owns heads {2c, 2c+1} (128 of the
1024 qkv feature dims). Each core computes Q/K/V for its head pair over the
full sequence, runs attention per (batch, head), then the chip reshards with
two AllToAlls per batch (one per 1024-token half) so core c ends up with the
full 1024 attention features for tokens {half*1024 + 128c ..+128} of every
batch. Each core applies the full W_proj to its token slices and the host
concatenates the shards.

Key engine-level structure (what makes this fast):
 - Scores S^T = K Q^T per head are K=64 matmuls; the two heads use PE row
   tiles (0,0)/(64,0) and run concurrently -> full PE utilization.
 - exp on the ScalarEngine (the true bottleneck: ~1 el/cycle/lane @1.2GHz,
   ~294us/core total). Emission interleaves scores matmul pairs with ns-
   budgeted chunks of PV/QKV/proj matmuls so the in-order engine streams
   stay paced to the exp cadence and the ACT engine is never starved.
 - PV is restructured: stationary = P^T tile [128 ktok, 128 q] bf16 (the exp
   output), moving = [V | 1] [128 ktok, 65] bf16. Out = [128 q, 65] f32
   accumulated over 16 k-tiles. M=128 means full PE columns (the old layout
   wasted half the array on M=65), and column 64 of the output is the softmax
   denominator as a per-partition scalar, so the divide is a DVE
   reciprocal + tensor_scalar_mul, no partition broadcast needed.
 - O comes out in natural [token, feature] layout; the [feat, token] layout
   needed for the A2A/proj is produced by PE-mode transposes (4 per PSUM
   bank) + one DVE copy per q-chunk -- cheap, and keeps the SP DMA queue
   short.
 - Each batch reshards with two half-token AllToAlls so the last batch's
   projection overlaps the final collective (short tail).
"""

import os
import sys

import numpy as np

for _p in ("/opt/trn_rl_repo", "/root/.axon_site/_ro/trn_rl_repo"):
    if os.path.isdir(_p) and _p not in sys.path:
        sys.path.append(_p)

import ml_dtypes  # noqa: E402

B, N, C = 4, 2048, 1024
NUM_HEADS = 16
HEAD_DIM = C // NUM_HEADS  # 64
SCALE = HEAD_DIM**-0.5
NCORES = 8
P = 128  # SBUF partitions
QC = 512  # q-chunk (matmul free dim / PSUM bank)
NCC2 = 4  # 256-channel DoubleRow contraction chunks

BF16 = ml_dtypes.bfloat16
FP8 = ml_dtypes.float8_e4m3


DEBUG_DUMP = False


def build_attention_nc(NB: int = B, NQ: int = N, CH: int = C):
    """Build + compile the SPMD graph. NB batches of NQ tokens, CH channels.

    Every core runs the same graph; per-core behavior differs only through the
    per-core input shards (wq/wk/wv slices) and the AllToAlls.
    """
    import concourse.bass as bass
    import concourse.mybir as mybir
    import concourse.tile as tile
    from concourse import bacc

    f32 = mybir.dt.float32
    bf16 = mybir.dt.bfloat16
    fp8 = mybir.dt.float8e4
    DR = mybir.MatmulPerfMode.DoubleRow

    n_qc = NQ // QC  # q chunks per batch (4)
    n_kt = NQ // P  # k tiles per batch (16)
    n_cc = CH // P  # bf16 contraction chunks (8, for proj)
    NQW = QC // P  # PV q-windows per q-chunk (4)
    TPB = NQ // NCORES  # tokens per core per batch after reshard (256)
    HTOK = NQ // 2  # tokens per A2A half (1024)

    nc = bacc.Bacc("TRN2", target_bir_lowering=False, debug=False,
                   num_devices=NCORES)

    xT = nc.dram_tensor("xT", [CH, NB * NQ], bf16, kind="ExternalInput").ap()
    wq = nc.dram_tensor("wq", [CH, P], bf16, kind="ExternalInput").ap()
    wk = nc.dram_tensor("wk", [CH, P], bf16, kind="ExternalInput").ap()
    wv = nc.dram_tensor("wv", [CH, P], bf16, kind="ExternalInput").ap()
    wp = nc.dram_tensor("wp", [CH, CH], bf16, kind="ExternalInput").ap()
    bp = nc.dram_tensor("bp", [1, CH], f32, kind="ExternalInput").ap()
    out = nc.dram_tensor("out", [NB * TPB, CH], f32, kind="ExternalOutput").ap()
    if DEBUG_DUMP:
        dbg_qt = nc.dram_tensor("dbg_qt", [P, NQ], bf16,
                                kind="ExternalOutput").ap()
        dbg_kt = nc.dram_tensor("dbg_kt", [P, NQ], bf16,
                                kind="ExternalOutput").ap()
        dbg_v = nc.dram_tensor("dbg_v", [P, (NQ // P) * 130], bf16,
                               kind="ExternalOutput").ap()
        dbg_pt = nc.dram_tensor("dbg_pt", [P, 2 * (NQ // P) * QC], bf16,
                                kind="ExternalOutput").ap()
        dbg_ot = nc.dram_tensor("dbg_ot", [P, NQ], bf16,
                                kind="ExternalOutput").ap()
        dbg_a2a = nc.dram_tensor("dbg_a2a", [NCORES * P, P], bf16,
                                 kind="ExternalOutput").ap()
        dbg_onat = nc.dram_tensor("dbg_onat", [P, (QC // P) * P], bf16,
                                  kind="ExternalOutput").ap()

    from contextlib import ExitStack

    with tile.TileContext(nc) as tc, ExitStack() as ctx:
        const = ctx.enter_context(tc.tile_pool(name="const", bufs=1))
        xt_pool = ctx.enter_context(tc.tile_pool(name="xt", bufs=8))
        qkv_pool = ctx.enter_context(tc.tile_pool(name="qkv", bufs=2))
        pt_pool = ctx.enter_context(tc.tile_pool(name="pt", bufs=5))
        onat_pool = ctx.enter_context(tc.tile_pool(name="onat", bufs=4))
        ot_pool = ctx.enter_context(tc.tile_pool(name="ot", bufs=2))
        div_pool = ctx.enter_context(tc.tile_pool(name="div", bufs=2))
        at_pool = ctx.enter_context(tc.tile_pool(name="at", bufs=16))
        y_pool = ctx.enter_context(tc.tile_pool(name="y", bufs=2))
        dram = ctx.enter_context(tc.tile_pool(name="dram", bufs=1, space="DRAM"))
        # PSUM budget (8 banks): ST 2x2 + mm 2 + pv 1 + transpose 1
        ps_st = ctx.enter_context(tc.tile_pool(name="ps_st", bufs=2, space="PSUM"))
        ps_mm = ctx.enter_context(tc.tile_pool(name="ps_mm", bufs=2, space="PSUM"))
        ps_pv = ctx.enter_context(tc.tile_pool(name="ps_pv", bufs=1, space="PSUM"))
        ps_tr = ctx.enter_context(tc.tile_pool(name="ps_tr", bufs=1, space="PSUM"))

        # --- resident weights ---
        wq_sb = const.tile([P, n_cc, P], bf16, tag="wq")
        wk_sb = const.tile([P, n_cc, P], bf16, tag="wk")
        wv_sb = const.tile([P, n_cc, P], bf16, tag="wv")
        nc.sync.dma_start(wk_sb[:], wk.rearrange("(cc p) m -> p cc m", p=P))
        nc.sync.dma_start(wq_sb[:], wq.rearrange("(cc p) m -> p cc m", p=P))
        nc.sync.dma_start(wv_sb[:], wv.rearrange("(cc p) m -> p cc m", p=P))
        wp_sb = const.tile([P, n_cc, CH], bf16, tag="wp")
        bias_row = const.tile([1, CH], f32, tag="bias_row")
        bias_sb = const.tile([P, CH], f32, tag="bias")
        ident = const.tile([P, P], bf16, tag="ident")
        from concourse.masks import make_identity
        make_identity(nc, ident[:])

        a2a_in = [[None, None] for _ in range(NB)]
        a2a_out = [[None, None] for _ in range(NB)]
        for b in range(NB):
            for hf in range(2):
                a2a_in[b][hf] = dram.tile([NCORES * P, P], bf16,
                                          tag=f"a2a_in{b}_{hf}",
                                          name=f"a2a_in{b}_{hf}")
                a2a_out[b][hf] = dram.tile([NCORES * P, P], bf16,
                                           tag=f"a2a_out{b}_{hf}",
                                           name=f"a2a_out{b}_{hf}")

        def emit_xt(b, split=1):
            xts = [xt_pool.tile([P, NQ], bf16, tag="xt", name="xt_tile")
                   for _ in range(n_cc)]
            hw = NQ // split
            for hv in range(split):
                for cc in range(n_cc):
                    nc.sync.dma_start(
                        xts[cc][:, hv * hw:(hv + 1) * hw],
                        xT[cc * P:(cc + 1) * P,
                           b * NQ + hv * hw:b * NQ + (hv + 1) * hw])
            return xts

        # ---- generator-based PE work, pumped between scores/exp steps ----
        # Every yield value is the approximate TensorEngine-ns of the chunk
        # just emitted; pump() draws chunks against an ns budget so the
        # in-order PE stream stays paced to the ~1147ns/exp ACT cadence.

        def gen_w_chunk(xts, w_sb, dst, qc):
            """One 512-token chunk of a QT/KT-style projection."""
            qs = slice(qc * QC, (qc + 1) * QC)
            ps = ps_mm.tile([P, QC], f32, tag="mm", name="qk_ps")
            for cc in range(n_cc):
                nc.tensor.matmul(ps[:], w_sb[:, cc, :], xts[cc][:, qs],
                                 start=(cc == 0), stop=(cc == n_cc - 1))
                if cc % 2 == 1:
                    yield 525
            nc.vector.tensor_copy(dst[:, qs], ps[:])

        def gen_kt_all(xts, kt_sb):
            """Full KT [128 head-dims, NQ tokens]. scores(qc) for ANY qc reads
            every k-tile, so this must be fully emitted before the batch's
            first scores matmul."""
            for qc in range(n_qc):
                for _ in gen_w_chunk(xts, wk_sb, kt_sb, qc):
                    yield _

        def gen_qt_qc(xts, qt_sb, qc):
            """QT for one q-chunk (scores(qc) reads only its own q columns)."""
            for _ in gen_w_chunk(xts, wq_sb, qt_sb, qc):
                yield _

        def gen_qkv_v(xts, v_sb):
            """V in natural [tok, head-dim] layout + ones column per head."""
            nc.vector.memset(v_sb[:, :, :, 64:65], 1.0)
            for tt in range(n_kt):
                ts_ = slice(tt * P, (tt + 1) * P)
                vps = ps_mm.tile([P, P], f32, tag="mm", name="vps")
                for cc in range(n_cc):
                    nc.tensor.matmul(vps[:], xts[cc][:, ts_], wv_sb[:, cc, :],
                                     start=(cc == 0), stop=(cc == n_cc - 1))
                nc.vector.tensor_copy(
                    v_sb[:, tt, :, 0:64], vps.rearrange("p (g c) -> p g c", g=2))
                yield 530

        def gen_pv(qc, pt_pair, v_sb, ot_sb, dbg=False):
            """O[q, d] = P V per head; col 64 = softmax denominator.

            Stationary = P^T tiles [128 ktok, 128 q] bf16, moving = [V|1]
            [128 ktok, 65] bf16; full-M matmuls, N=65. Divide on DVE with the
            denominator as a per-partition scalar, then PE-mode transposes
            (4 into one PSUM bank) + one DVE copy into ot_sb [feat, q]."""
            onats = [onat_pool.tile([P, P], bf16, tag="onat", name="onat_t")
                     for _ in range(NQW)]
            for h in range(2):
                apv = ps_pv.tile([P, NQW, 65], f32, tag="pv", name="apv_t")
                # one qw chain at a time: a start=True matmul clears
                # has_written beyond its own region, so concurrently-started
                # chains in one PSUM bank lose each other's first tile
                for qw in range(NQW):
                    for kt in range(n_kt):
                        pt_h = pt_pair[0] if kt < n_kt // 2 else pt_pair[1]
                        kk = 2 * (kt % (n_kt // 2)) + h
                        nc.tensor.matmul(
                            apv[:, qw, :],
                            pt_h[:, kk, qw * P:(qw + 1) * P],
                            v_sb[:, kt, h, :],
                            start=(kt == 0), stop=(kt == n_kt - 1))
                        if kt % 12 == 11:
                            yield 430
                    # divide each q-window as soon as its chain stops, so
                    # only the last window's divide sits adjacent to the next
                    # wave's first matmul in the engine streams
                    rec = div_pool.tile([P, 1], f32, tag="rec", name="rec_t")
                    nc.vector.reciprocal(rec[:], apv[:, qw, 64:65])
                    nc.vector.tensor_scalar_mul(
                        onats[qw][:, 64 * h:64 * (h + 1)],
                        apv[:, qw, 0:64], rec[:])
                    yield 120
            if dbg:
                for qw in range(NQW):
                    nc.sync.dma_start(dbg_onat[:, qw * P:(qw + 1) * P],
                                      onats[qw][:])
            trp = ps_tr.tile([P, NQW, P], bf16, tag="tr", name="tr_t")
            for qw in range(NQW):
                nc.tensor.transpose(trp[:, qw, :], onats[qw][:], ident[:])
                yield 290
            nc.vector.tensor_copy(
                ot_sb.rearrange("p (c w q) -> p c w q",
                                c=n_qc, w=NQW)[:, qc], trp[:])
            yield 250

        def gen_proj(b, hf, ats):
            """W_proj + bias for this core's 128 tokens of (batch b, half hf)."""
            for oc in range(CH // QC):
                ocs = slice(oc * QC, (oc + 1) * QC)
                yps = ps_mm.tile([P, QC], f32, tag="mm", name="yps_t")
                for cc in range(n_cc):
                    nc.tensor.matmul(yps[:], ats[cc][:], wp_sb[:, cc, ocs],
                                     start=(cc == 0), stop=(cc == n_cc - 1))
                    if cc % 2 == 1:
                        yield 525
                y_sb = y_pool.tile([P, QC], f32, tag="y", name="y_tile")
                nc.vector.tensor_add(y_sb[:], yps[:], bias_sb[:, ocs])
                nc.sync.dma_start(
                    out[b * TPB + hf * P:b * TPB + (hf + 1) * P, ocs], y_sb[:])

        # ---- work queue pump ----
        # Background PE work (PV/QKV/proj matmul chains) is emitted through
        # generators pumped between scores/exp steps, so the TensorEngine has
        # dense work while the ScalarEngine grinds through exp (the
        # bottleneck), instead of the engines ping-ponging phase by phase.
        from collections import deque
        work = deque()   # current-batch critical gens: round-robin
        slack = deque()  # next-batch QKV + proj: strictly ordered AFTER work

        def pump(budget_ns):
            # Two tiers. `work` (this batch's PV + QT leftovers) round-robins:
            # consecutive chunks of one generator often have an internal
            # cross-engine dependency (PV's wave divide -> next wave's first
            # matmul), and interleaving keeps the in-order PE stream free of
            # head-of-line stalls at those seams. `slack` (next-batch QKV,
            # proj) is drawn only when `work` is empty, preserving FIFO order
            # relative to this batch's chains -- next-batch tile-slot WARs
            # (xt/at reuse) then always point backwards in the engine
            # streams, which is deadlock-free by construction.
            acc = 0
            while acc < budget_ns:
                if work:
                    try:
                        acc += next(work[0])
                        work.rotate(-1)
                    except StopIteration:
                        work.popleft()
                        continue
                    if len(work) == 1 and slack:
                        # a single critical gen (PV) has nothing to round-
                        # robin with; alternate its chunks with slack so its
                        # back-to-back weight loads get background slots (all
                        # current-batch chains are emitted by this point, so
                        # slack's slot-reuse WARs still point backwards)
                        try:
                            acc += next(slack[0])
                        except StopIteration:
                            slack.popleft()
                elif slack:
                    try:
                        acc += next(slack[0])
                    except StopIteration:
                        slack.popleft()
                else:
                    break

        def finish(g):
            """Fully emit generator g (it may be anywhere in the queue)."""
            if g is None:
                return
            for _ in g:
                pass

        def drain():
            while work or slack:
                pump(1 << 30)

        def emit_scores(qc, qt_sb, kt_sb, pump_ns=850):
            """S^T = K Q^T row-tiled head pair -> exp -> pt (bf16).

            Each kt step costs the ACT engine ~1147ns; the scores pair is
            ~270ns of PE, so pump ~700ns of background PE work per step --
            deliberately under-subscribed so a score pair is never queued
            behind excess backlog in the in-order PE stream; the leftover
            drains in the exp windows after each q-chunk's last scores."""
            qs = slice(qc * QC, (qc + 1) * QC)
            # pt in half-q-chunk tiles (5-buf pool): the odd-stride slot
            # rotation gives every recycled slot a >=1.5 q-chunk lag, so its
            # previous readers (PV of an older q-chunk) are finished and the
            # exp chain never WAR-stalls behind PV
            pt_a = pt_pool.tile([P, n_kt, QC], bf16, tag="pt", name="pt_a")
            pt_b = None
            for kt in range(n_kt):
                if kt == n_kt // 2:
                    pt_b = pt_pool.tile([P, n_kt, QC], bf16, tag="pt",
                                        name="pt_b")
                pt_h = pt_a if kt < n_kt // 2 else pt_b
                kk = 2 * (kt % (n_kt // 2))
                ks = slice(kt * P, (kt + 1) * P)
                st = ps_st.tile([P, 2, QC], f32, tag="st", name="st_tile")
                for h in range(2):
                    hs = slice(64 * h, 64 * (h + 1))
                    nc.tensor.matmul(st[:, h, :], kt_sb[hs, ks],
                                     qt_sb[hs, qs])
                nc.scalar.activation(pt_h[:, kk:kk + 2, :], st[:],
                                     mybir.ActivationFunctionType.Exp,
                                     scale=SCALE)
                pump(pump_ns)
            return pt_a, pt_b

        def emit_a2a(b, hf, ot_sb):
            nc.gpsimd.dma_start(
                a2a_in[b][hf].rearrange("(j p) t -> p j t", p=P),
                ot_sb[:, hf * HTOK:(hf + 1) * HTOK].rearrange(
                    "p (j t) -> p j t", j=NCORES))
            nc.gpsimd.collective_compute(
                "AllToAll", mybir.AluOpType.bypass,
                replica_groups=[list(range(NCORES))],
                ins=[a2a_in[b][hf][:].opt()], outs=[a2a_out[b][hf][:].opt()])

        def emit_proj_loads(b, hf):
            ats = []
            for cc in range(n_cc):
                at = at_pool.tile([P, P], bf16, tag="at", name="at_tile")
                nc.sync.dma_start(at[:], a2a_out[b][hf][cc * P:(cc + 1) * P, :])
                ats.append(at)
            return ats

        # ---- main program ----
        # Batch loop is software-pipelined one stage deep: batch b-1's last
        # PV chunk and second AllToAll are carried into batch b's scores
        # region, so the PE keeps working through the exp of the next batch
        # and the ACT engine never waits on a batch boundary.
        xts = emit_xt(0, split=4)
        qt_sb = qkv_pool.tile([P, NQ], bf16, tag="qt")
        kt_sb = qkv_pool.tile([P, NQ], bf16, tag="kt")
        v_sb = qkv_pool.tile([P, n_kt, 2, 65], bf16, tag="v", bufs=3)
        # batch 0 bootstrap, ordered so the first scores pair can execute
        # as early as possible: KT token-chunk 0 and QT chunk 0 first (the
        # only inputs of the first score tiles), then the rest of KT; QT
        # chunks 1-3 and V interleave into the exp cadence
        finish(gen_w_chunk(xts, wk_sb, kt_sb, 0))
        g_qts = [gen_qt_qc(xts, qt_sb, qc) for qc in range(n_qc)]
        finish(g_qts[0])
        for qc in range(1, n_qc):
            finish(gen_w_chunk(xts, wk_sb, kt_sb, qc))
        g_kt = None
        g_v = gen_qkv_v(xts, v_sb)
        # all non-PV gens go through `slack` so the `work` tier is PV-only
        # (the singleton-slack interleave in pump() relies on that)
        slack.extend(g_qts[1:])
        slack.append(g_v)
        # W_proj + bias aren't needed until the first projection
        nc.sync.dma_start(wp_sb[:], wp.rearrange("(cc p) m -> p cc m", p=P))
        nc.sync.dma_start(bias_row[:], bp[:, :])
        nc.gpsimd.partition_broadcast(bias_sb[:], bias_row[:])

        g_pv_carry = g_pv_carry2 = None
        prev_ot = None
        for b in range(NB):
            # this batch's full KT must be emitted before its first scores
            # matmul (normally already drained by batch b-1's pumps)
            finish(g_kt)
            if b + 1 < NB:
                next_xts = emit_xt(b + 1)
                nqt = qkv_pool.tile([P, NQ], bf16, tag="qt", name="nqt")
                nkt = qkv_pool.tile([P, NQ], bf16, tag="kt", name="nkt")
                nv = qkv_pool.tile([P, n_kt, 2, 65], bf16, tag="v", name="nv", bufs=3)
            if b + 1 < NB:
                n_kt_g = gen_kt_all(next_xts, nkt)
                n_qts = [gen_qt_qc(next_xts, nqt, qc) for qc in range(n_qc)]
                n_v = gen_qkv_v(next_xts, nv)
                slack.append(n_kt_g)
                slack.append(n_v)
                slack.extend(n_qts)
            if b > 0:
                # proj is the least time-critical work (nothing reads its
                # output until the kernel end): keep it at the queue back as
                # the slack absorber
                ats0 = emit_proj_loads(b - 1, 0)
                slack.append(gen_proj(b - 1, 0, ats0))

            ot_sb = ot_pool.tile([P, NQ], bf16, tag="ot")
            pts = [None] * n_qc
            pv_gens = [None] * n_qc
            for qc in range(n_qc):
                # this batch's QT for qc must be fully emitted before the
                # scores that read it (normally already drained by pumps)
                finish(g_qts[qc])
                pts[qc] = emit_scores(qc, qt_sb, kt_sb)
                if qc == 0:
                    if DEBUG_DUMP and b == 0:
                        dpv = dbg_pt.rearrange("p (j q) -> p j q",
                                               j=2 * (NQ // P))
                        nc.sync.dma_start(dpv[:, 0:n_kt], pts[0][0][:])
                        nc.sync.dma_start(dpv[:, n_kt:2 * n_kt], pts[0][1][:])
                    if b > 0:
                        # batch b-1's tail: its last PVs ran during exp(b, 0)
                        finish(g_pv_carry2)
                        finish(g_pv_carry)
                        if DEBUG_DUMP and b == 1:
                            nc.sync.dma_start(dbg_ot[:, :], prev_ot[:])
                        emit_a2a(b - 1, 1, prev_ot)
                        ats1 = emit_proj_loads(b - 1, 1)
                        slack.append(gen_proj(b - 1, 1, ats1))
                # queue this q-chunk's PV at the FRONT of the work queue: it
                # is the dep-critical background work (its pt slots must
                # recycle for later exps) and fills the very next exp window
                finish(g_v)
                pv_gens[qc] = gen_pv(qc, pts[qc], v_sb, ot_sb,
                                     dbg=DEBUG_DUMP and b == 0 and qc == 0)
                work.appendleft(pv_gens[qc])
            finish(pv_gens[0])
            finish(pv_gens[1])
            emit_a2a(b, 0, ot_sb)
            if b == NB - 1:
                # the last batch's first-half projection can ride the exp
                # drain window instead of the serial tail
                ats_l0 = emit_proj_loads(b, 0)
                slack.append(gen_proj(b, 0, ats_l0))
            # force ~16us of backlog emission here: it executes during the
            # last q-chunk's exp drain and the batch boundary, where the PE
            # would otherwise idle (and keeps proj from piling up at the end)
            pump(16000)
            if DEBUG_DUMP and b == 0:
                nc.sync.dma_start(dbg_qt[:, :], qt_sb[:])
                nc.sync.dma_start(dbg_kt[:, :], kt_sb[:])
                nc.sync.dma_start(
                    dbg_v.rearrange("p (t g c) -> p t g c", t=NQ // P, g=2),
                    v_sb[:])
                nc.sync.dma_start(dbg_a2a[:, :], a2a_out[0][0][:])
            g_pv_carry2 = pv_gens[2]
            g_pv_carry = pv_gens[3]
            prev_ot = ot_sb
            if b + 1 < NB:
                qt_sb, kt_sb, v_sb = nqt, nkt, nv
                g_kt, g_qts, g_v = n_kt_g, n_qts, n_v
                xts = next_xts

        # final tail: finish the last PVs, fire the last collective, then
        # drain the proj backlog while it flies
        finish(g_pv_carry2)
        finish(g_pv_carry)
        emit_a2a(NB - 1, 1, prev_ot)
        ats1 = emit_proj_loads(NB - 1, 1)
        slack.append(gen_proj(NB - 1, 1, ats1))
        drain()

    nc.compile()
    return nc


def make_in_maps(x, W_qkv, W_proj, b_proj, NB=B, NQ=N, CH=C):
    """Shard the full inputs into one input map per core."""
    xT = np.ascontiguousarray(x.reshape(NB * NQ, CH).T).astype(BF16)
    wp = np.ascontiguousarray(W_proj).astype(BF16)
    bp = np.ascontiguousarray(b_proj[None, :]).astype(np.float32)
    in_maps = []
    for c in range(NCORES):
        cs = slice(P * c, P * (c + 1))
        in_maps.append({
            "xT": xT,
            "wq": np.ascontiguousarray(W_qkv[:, cs]).astype(BF16),
            "wk": np.ascontiguousarray(W_qkv[:, CH:][:, cs]).astype(BF16),
            "wv": np.ascontiguousarray(W_qkv[:, 2 * CH:][:, cs]).astype(BF16),
            "wp": wp,
            "bp": bp,
        })
    return in_maps


def assemble_output(results, NB=B, NQ=N, CH=C):
    """Concatenate the per-core token shards into the full output.

    Core c's out rows [b*256 + hf*128 .. +128] hold tokens
    [hf*1024 + 128c .. +128] of batch b."""
    TPB = NQ // NCORES
    full = np.empty((NB, NQ, CH), dtype=np.float32)
    for c in range(NCORES):
        y = np.asarray(results[c]["out"], dtype=np.float32)
        for b in range(NB):
            for hf in range(2):
                full[b, hf * (NQ // 2) + P * c:hf * (NQ // 2) + P * (c + 1), :] = \
                    y[b * TPB + hf * P:b * TPB + (hf + 1) * P]
    return full


_compiled_nc = None


def kernel(x, W_qkv, W_proj, b_proj):
    global _compiled_nc
    x = np.asarray(x, dtype=np.float32)
    W_qkv = np.asarray(W_qkv, dtype=np.float32)
    W_proj = np.asarray(W_proj, dtype=np.float32)
    b_proj = np.asarray(b_proj, dtype=np.float32)

    if _compiled_nc is None:
        _compiled_nc = build_attention_nc()

    from concourse.bass_utils import run_bass_kernel_spmd

    in_maps = make_in_maps(x, W_qkv, W_proj, b_proj)
    res = run_bass_kernel_spmd(_compiled_nc, in_maps,
                               core_ids=list(range(NCORES)))
    return assemble_output(res.results)


# revision 48
# speedup vs baseline: 1.1142x; 1.1142x over previous
"""Distributed multi-head attention kernel for one TRN2 chip (8 NeuronCores).

Problem: x[4, 2048, 1024] -> qkv Linear(1024, 3072, bias=False) -> 16-head
softmax attention -> proj Linear(1024, 1024) + bias.

Sharding: tensor-parallel over heads. Core c owns heads {2c, 2c+1} (128 of the
1024 qkv feature dims). Each core computes Q/K/V for its head pair over the
full sequence, runs attention per (batch, head), then the chip reshards with
two AllToAlls per batch (one per 1024-token half) so core c ends up with the
full 1024 attention features for tokens {half*1024 + 128c ..+128} of every
batch. Each core applies the full W_proj to its token slices and the host
concatenates the shards.

Key engine-level structure (what makes this fast):
 - Scores S^T = K Q^T per head are K=64 matmuls; the two heads use PE row
   tiles (0,0)/(64,0) and run concurrently -> full PE utilization.
 - exp on the ScalarEngine (the true bottleneck: ~1 el/cycle/lane @1.2GHz,
   ~294us/core total). Emission interleaves scores matmul pairs with ns-
   budgeted chunks of PV/QKV/proj matmuls so the in-order engine streams
   stay paced to the exp cadence and the ACT engine is never starved.
 - PV is restructured: stationary = P^T tile [128 ktok, 128 q] bf16 (the exp
   output), moving = [V | 1] [128 ktok, 65] bf16. Out = [128 q, 65] f32
   accumulated over 16 k-tiles. M=128 means full PE columns (the old layout
   wasted half the array on M=65), and column 64 of the output is the softmax
   denominator as a per-partition scalar, so the divide is a DVE
   reciprocal + tensor_scalar_mul, no partition broadcast needed.
 - O comes out in natural [token, feature] layout; the [feat, token] layout
   needed for the A2A/proj is produced by PE-mode transposes (4 per PSUM
   bank) + one DVE copy per q-chunk -- cheap, and keeps the SP DMA queue
   short.
 - Each batch reshards with two half-token AllToAlls so the last batch's
   projection overlaps the final collective (short tail).
"""

import os
import sys

import numpy as np

for _p in ("/opt/trn_rl_repo", "/root/.axon_site/_ro/trn_rl_repo"):
    if os.path.isdir(_p) and _p not in sys.path:
        sys.path.append(_p)

import ml_dtypes  # noqa: E402

B, N, C = 4, 2048, 1024
NUM_HEADS = 16
HEAD_DIM = C // NUM_HEADS  # 64
SCALE = HEAD_DIM**-0.5
NCORES = 8
P = 128  # SBUF partitions
QC = 512  # q-chunk (matmul free dim / PSUM bank)
NCC2 = 4  # 256-channel DoubleRow contraction chunks

BF16 = ml_dtypes.bfloat16
FP8 = ml_dtypes.float8_e4m3


DEBUG_DUMP = False


def build_attention_nc(NB: int = B, NQ: int = N, CH: int = C):
    """Build + compile the SPMD graph. NB batches of NQ tokens, CH channels.

    Every core runs the same graph; per-core behavior differs only through the
    per-core input shards (wq/wk/wv slices) and the AllToAlls.
    """
    import concourse.bass as bass
    import concourse.mybir as mybir
    import concourse.tile as tile
    from concourse import bacc

    f32 = mybir.dt.float32
    bf16 = mybir.dt.bfloat16
    fp8 = mybir.dt.float8e4
    DR = mybir.MatmulPerfMode.DoubleRow

    n_qc = NQ // QC  # q chunks per batch (4)
    n_kt = NQ // P  # k tiles per batch (16)
    n_cc = CH // P  # bf16 contraction chunks (8, for proj)
    NQW = QC // P  # PV q-windows per q-chunk (4)
    TPB = NQ // NCORES  # tokens per core per batch after reshard (256)
    HTOK = NQ // 2  # tokens per A2A half (1024)

    nc = bacc.Bacc("TRN2", target_bir_lowering=False, debug=False,
                   num_devices=NCORES)

    xT = nc.dram_tensor("xT", [CH, NB * NQ], bf16, kind="ExternalInput").ap()
    wq = nc.dram_tensor("wq", [CH, P], bf16, kind="ExternalInput").ap()
    wk = nc.dram_tensor("wk", [CH, P], bf16, kind="ExternalInput").ap()
    wv = nc.dram_tensor("wv", [CH, P], bf16, kind="ExternalInput").ap()
    wp = nc.dram_tensor("wp", [CH, CH], bf16, kind="ExternalInput").ap()
    bp = nc.dram_tensor("bp", [1, CH], f32, kind="ExternalInput").ap()
    out = nc.dram_tensor("out", [NB * TPB, CH], f32, kind="ExternalOutput").ap()
    if DEBUG_DUMP:
        dbg_qt = nc.dram_tensor("dbg_qt", [P, NQ], bf16,
                                kind="ExternalOutput").ap()
        dbg_kt = nc.dram_tensor("dbg_kt", [P, NQ], bf16,
                                kind="ExternalOutput").ap()
        dbg_v = nc.dram_tensor("dbg_v", [P, (NQ // P) * 130], bf16,
                               kind="ExternalOutput").ap()
        dbg_pt = nc.dram_tensor("dbg_pt", [P, 2 * (NQ // P) * QC], bf16,
                                kind="ExternalOutput").ap()
        dbg_ot = nc.dram_tensor("dbg_ot", [P, NQ], bf16,
                                kind="ExternalOutput").ap()
        dbg_a2a = nc.dram_tensor("dbg_a2a", [NCORES * P, P], bf16,
                                 kind="ExternalOutput").ap()
        dbg_onat = nc.dram_tensor("dbg_onat", [P, (QC // P) * P], bf16,
                                  kind="ExternalOutput").ap()

    from contextlib import ExitStack

    with tile.TileContext(nc) as tc, ExitStack() as ctx:
        const = ctx.enter_context(tc.tile_pool(name="const", bufs=1))
        xt_pool = ctx.enter_context(tc.tile_pool(name="xt", bufs=8))
        qkv_pool = ctx.enter_context(tc.tile_pool(name="qkv", bufs=2))
        pt_pool = ctx.enter_context(tc.tile_pool(name="pt", bufs=5))
        onat_pool = ctx.enter_context(tc.tile_pool(name="onat", bufs=4))
        ot_pool = ctx.enter_context(tc.tile_pool(name="ot", bufs=2))
        div_pool = ctx.enter_context(tc.tile_pool(name="div", bufs=2))
        at_pool = ctx.enter_context(tc.tile_pool(name="at", bufs=16))
        y_pool = ctx.enter_context(tc.tile_pool(name="y", bufs=2))
        dram = ctx.enter_context(tc.tile_pool(name="dram", bufs=1, space="DRAM"))
        # PSUM budget (8 banks): ST 2x2 + mm 2 + pv 1 + transpose 1
        ps_st = ctx.enter_context(tc.tile_pool(name="ps_st", bufs=2, space="PSUM"))
        ps_mm = ctx.enter_context(tc.tile_pool(name="ps_mm", bufs=2, space="PSUM"))
        ps_pv = ctx.enter_context(tc.tile_pool(name="ps_pv", bufs=1, space="PSUM"))
        ps_tr = ctx.enter_context(tc.tile_pool(name="ps_tr", bufs=1, space="PSUM"))

        # --- resident weights ---
        wq_sb = const.tile([P, n_cc, P], bf16, tag="wq")
        wk_sb = const.tile([P, n_cc, P], bf16, tag="wk")
        wv_sb = const.tile([P, n_cc, P], bf16, tag="wv")
        nc.sync.dma_start(wk_sb[:], wk.rearrange("(cc p) m -> p cc m", p=P))
        nc.sync.dma_start(wq_sb[:], wq.rearrange("(cc p) m -> p cc m", p=P))
        nc.sync.dma_start(wv_sb[:], wv.rearrange("(cc p) m -> p cc m", p=P))
        wp_sb = const.tile([P, n_cc, CH], bf16, tag="wp")
        bias_row = const.tile([1, CH], f32, tag="bias_row")
        bias_sb = const.tile([P, CH], f32, tag="bias")
        ident = const.tile([P, P], bf16, tag="ident")
        from concourse.masks import make_identity
        make_identity(nc, ident[:])

        a2a_in = [[None, None] for _ in range(NB)]
        a2a_out = [[None, None] for _ in range(NB)]
        for b in range(NB):
            for hf in range(2):
                if b == NB - 1 and hf == 1:
                    continue
                a2a_in[b][hf] = dram.tile([NCORES * P, P], bf16,
                                          tag=f"a2a_in{b}_{hf}",
                                          name=f"a2a_in{b}_{hf}")
                a2a_out[b][hf] = dram.tile([NCORES * P, P], bf16,
                                           tag=f"a2a_out{b}_{hf}",
                                           name=f"a2a_out{b}_{hf}")
        # the last batch's second half reshards as two quarter collectives
        # (one per q-chunk) so only a 256KB collective + a 64-token
        # projection sit in the serial kernel tail
        QT_TOK = QC // NCORES  # 64 tokens per core per quarter
        a2a_qin = [dram.tile([NCORES * P, QT_TOK], bf16, tag=f"a2a_qin{q}",
                             name=f"a2a_qin{q}") for q in range(2)]
        a2a_qout = [dram.tile([NCORES * P, QT_TOK], bf16, tag=f"a2a_qout{q}",
                              name=f"a2a_qout{q}") for q in range(2)]

        def emit_xt(b, split=1):
            xts = [xt_pool.tile([P, NQ], bf16, tag="xt", name="xt_tile")
                   for _ in range(n_cc)]
            hw = NQ // split
            for hv in range(split):
                for cc in range(n_cc):
                    nc.sync.dma_start(
                        xts[cc][:, hv * hw:(hv + 1) * hw],
                        xT[cc * P:(cc + 1) * P,
                           b * NQ + hv * hw:b * NQ + (hv + 1) * hw])
            return xts

        # ---- generator-based PE work, pumped between scores/exp steps ----
        # Every yield value is the approximate TensorEngine-ns of the chunk
        # just emitted; pump() draws chunks against an ns budget so the
        # in-order PE stream stays paced to the ~1147ns/exp ACT cadence.

        def gen_w_chunk(xts, w_sb, dst, qc):
            """One 512-token chunk of a QT/KT-style projection."""
            qs = slice(qc * QC, (qc + 1) * QC)
            ps = ps_mm.tile([P, QC], f32, tag="mm", name="qk_ps")
            for cc in range(n_cc):
                nc.tensor.matmul(ps[:], w_sb[:, cc, :], xts[cc][:, qs],
                                 start=(cc == 0), stop=(cc == n_cc - 1))
                if cc % 2 == 1:
                    yield 525
            nc.vector.tensor_copy(dst[:, qs], ps[:])

        def gen_kt_all(xts, kt_sb):
            """Full KT [128 head-dims, NQ tokens]. scores(qc) for ANY qc reads
            every k-tile, so this must be fully emitted before the batch's
            first scores matmul."""
            for qc in range(n_qc):
                for _ in gen_w_chunk(xts, wk_sb, kt_sb, qc):
                    yield _

        def gen_qt_qc(xts, qt_sb, qc):
            """QT for one q-chunk (scores(qc) reads only its own q columns)."""
            for _ in gen_w_chunk(xts, wq_sb, qt_sb, qc):
                yield _

        def gen_qkv_v(xts, v_sb):
            """V in natural [tok, head-dim] layout + ones column per head."""
            nc.vector.memset(v_sb[:, :, :, 64:65], 1.0)
            for tt in range(n_kt):
                ts_ = slice(tt * P, (tt + 1) * P)
                vps = ps_mm.tile([P, P], f32, tag="mm", name="vps")
                for cc in range(n_cc):
                    nc.tensor.matmul(vps[:], xts[cc][:, ts_], wv_sb[:, cc, :],
                                     start=(cc == 0), stop=(cc == n_cc - 1))
                nc.vector.tensor_copy(
                    v_sb[:, tt, :, 0:64], vps.rearrange("p (g c) -> p g c", g=2))
                yield 530

        def gen_pv(qc, pt_pair, v_sb, ot_sb, dbg=False):
            """O[q, d] = P V per head; col 64 = softmax denominator.

            Stationary = P^T tiles [128 ktok, 128 q] bf16, moving = [V|1]
            [128 ktok, 65] bf16; full-M matmuls, N=65. Divide on DVE with the
            denominator as a per-partition scalar, then PE-mode transposes
            (4 into one PSUM bank) + one DVE copy into ot_sb [feat, q]."""
            onats = [onat_pool.tile([P, P], bf16, tag="onat", name="onat_t")
                     for _ in range(NQW)]
            for h in range(2):
                apv = ps_pv.tile([P, NQW, 65], f32, tag="pv", name="apv_t")
                # one qw chain at a time: a start=True matmul clears
                # has_written beyond its own region, so concurrently-started
                # chains in one PSUM bank lose each other's first tile
                for qw in range(NQW):
                    for kt in range(n_kt):
                        pt_h = pt_pair[0] if kt < n_kt // 2 else pt_pair[1]
                        kk = 2 * (kt % (n_kt // 2)) + h
                        nc.tensor.matmul(
                            apv[:, qw, :],
                            pt_h[:, kk, qw * P:(qw + 1) * P],
                            v_sb[:, kt, h, :],
                            start=(kt == 0), stop=(kt == n_kt - 1))
                        if kt % 12 == 11:
                            yield 430
                    # divide each q-window as soon as its chain stops, so
                    # only the last window's divide sits adjacent to the next
                    # wave's first matmul in the engine streams
                    rec = div_pool.tile([P, 1], f32, tag="rec", name="rec_t")
                    nc.vector.reciprocal(rec[:], apv[:, qw, 64:65])
                    nc.vector.tensor_scalar_mul(
                        onats[qw][:, 64 * h:64 * (h + 1)],
                        apv[:, qw, 0:64], rec[:])
                    yield 120
            if dbg:
                for qw in range(NQW):
                    nc.sync.dma_start(dbg_onat[:, qw * P:(qw + 1) * P],
                                      onats[qw][:])
            trp = ps_tr.tile([P, NQW, P], bf16, tag="tr", name="tr_t")
            for qw in range(NQW):
                nc.tensor.transpose(trp[:, qw, :], onats[qw][:], ident[:])
                yield 290
            nc.vector.tensor_copy(
                ot_sb.rearrange("p (c w q) -> p c w q",
                                c=n_qc, w=NQW)[:, qc], trp[:])
            yield 250

        def gen_proj(b, hf, ats):
            """W_proj + bias for this core's 128 tokens of (batch b, half hf)."""
            for oc in range(CH // QC):
                ocs = slice(oc * QC, (oc + 1) * QC)
                yps = ps_mm.tile([P, QC], f32, tag="mm", name="yps_t")
                for cc in range(n_cc):
                    nc.tensor.matmul(yps[:], ats[cc][:], wp_sb[:, cc, ocs],
                                     start=(cc == 0), stop=(cc == n_cc - 1))
                    if cc % 2 == 1:
                        yield 525
                y_sb = y_pool.tile([P, QC], f32, tag="y", name="y_tile")
                nc.vector.tensor_add(y_sb[:], yps[:], bias_sb[:, ocs])
                nc.sync.dma_start(
                    out[b * TPB + hf * P:b * TPB + (hf + 1) * P, ocs], y_sb[:])

        # ---- work queue pump ----
        # Background PE work (PV/QKV/proj matmul chains) is emitted through
        # generators pumped between scores/exp steps, so the TensorEngine has
        # dense work while the ScalarEngine grinds through exp (the
        # bottleneck), instead of the engines ping-ponging phase by phase.
        from collections import deque
        work = deque()   # current-batch critical gens: round-robin
        slack = deque()  # next-batch QKV + proj: strictly ordered AFTER work

        def pump(budget_ns):
            # Two tiers. `work` (this batch's PV + QT leftovers) round-robins:
            # consecutive chunks of one generator often have an internal
            # cross-engine dependency (PV's wave divide -> next wave's first
            # matmul), and interleaving keeps the in-order PE stream free of
            # head-of-line stalls at those seams. `slack` (next-batch QKV,
            # proj) is drawn only when `work` is empty, preserving FIFO order
            # relative to this batch's chains -- next-batch tile-slot WARs
            # (xt/at reuse) then always point backwards in the engine
            # streams, which is deadlock-free by construction.
            acc = 0
            while acc < budget_ns:
                if work:
                    try:
                        acc += next(work[0])
                        work.rotate(-1)
                    except StopIteration:
                        work.popleft()
                        continue
                    if len(work) == 1 and slack:
                        # a single critical gen (PV) has nothing to round-
                        # robin with; alternate its chunks with slack so its
                        # back-to-back weight loads get background slots (all
                        # current-batch chains are emitted by this point, so
                        # slack's slot-reuse WARs still point backwards)
                        try:
                            acc += next(slack[0])
                        except StopIteration:
                            slack.popleft()
                elif slack:
                    try:
                        acc += next(slack[0])
                    except StopIteration:
                        slack.popleft()
                else:
                    break

        def finish(g):
            """Fully emit generator g (it may be anywhere in the queue)."""
            if g is None:
                return
            for _ in g:
                pass

        def drain():
            while work or slack:
                pump(1 << 30)

        def emit_scores(qc, qt_sb, kt_sb, pump_ns=850):
            """S^T = K Q^T row-tiled head pair -> exp -> pt (bf16).

            Each kt step costs the ACT engine ~1147ns; the scores pair is
            ~270ns of PE, so pump ~700ns of background PE work per step --
            deliberately under-subscribed so a score pair is never queued
            behind excess backlog in the in-order PE stream; the leftover
            drains in the exp windows after each q-chunk's last scores."""
            qs = slice(qc * QC, (qc + 1) * QC)
            # pt in half-q-chunk tiles (5-buf pool): the odd-stride slot
            # rotation gives every recycled slot a >=1.5 q-chunk lag, so its
            # previous readers (PV of an older q-chunk) are finished and the
            # exp chain never WAR-stalls behind PV
            pt_a = pt_pool.tile([P, n_kt, QC], bf16, tag="pt", name="pt_a")
            pt_b = None
            for kt in range(n_kt):
                if kt == n_kt // 2:
                    pt_b = pt_pool.tile([P, n_kt, QC], bf16, tag="pt",
                                        name="pt_b")
                pt_h = pt_a if kt < n_kt // 2 else pt_b
                kk = 2 * (kt % (n_kt // 2))
                ks = slice(kt * P, (kt + 1) * P)
                st = ps_st.tile([P, 2, QC], f32, tag="st", name="st_tile")
                for h in range(2):
                    hs = slice(64 * h, 64 * (h + 1))
                    nc.tensor.matmul(st[:, h, :], kt_sb[hs, ks],
                                     qt_sb[hs, qs])
                nc.scalar.activation(pt_h[:, kk:kk + 2, :], st[:],
                                     mybir.ActivationFunctionType.Exp,
                                     scale=SCALE)
                pump(pump_ns)
            return pt_a, pt_b

        def emit_a2a(b, hf, ot_sb):
            nc.gpsimd.dma_start(
                a2a_in[b][hf].rearrange("(j p) t -> p j t", p=P),
                ot_sb[:, hf * HTOK:(hf + 1) * HTOK].rearrange(
                    "p (j t) -> p j t", j=NCORES))
            nc.gpsimd.collective_compute(
                "AllToAll", mybir.AluOpType.bypass,
                replica_groups=[list(range(NCORES))],
                ins=[a2a_in[b][hf][:].opt()], outs=[a2a_out[b][hf][:].opt()])

        def emit_a2a_quarter(q, ot_sb):
            """Reshard q-chunk (2 + q) of the last batch: core c gets tokens
            [(2 + q) * 512 + 64c .. +64)."""
            qbase = (2 + q) * QC
            nc.gpsimd.dma_start(
                a2a_qin[q].rearrange("(j p) t -> p j t", p=P),
                ot_sb[:, qbase:qbase + QC].rearrange(
                    "p (j t) -> p j t", j=NCORES))
            nc.gpsimd.collective_compute(
                "AllToAll", mybir.AluOpType.bypass,
                replica_groups=[list(range(NCORES))],
                ins=[a2a_qin[q][:].opt()], outs=[a2a_qout[q][:].opt()])

        def gen_proj_quarter(q, ats):
            """W_proj + bias for this core's 64 tokens of quarter q."""
            row0 = (NB - 1) * TPB + P + q * QT_TOK
            for oc in range(CH // QC):
                ocs = slice(oc * QC, (oc + 1) * QC)
                yps = ps_mm.tile([P, QC], f32, tag="mm", name="yq_t")
                for cc in range(n_cc):
                    nc.tensor.matmul(yps[:QT_TOK, :], ats[cc][:, :QT_TOK],
                                     wp_sb[:, cc, ocs],
                                     start=(cc == 0), stop=(cc == n_cc - 1))
                    if cc % 2 == 1:
                        yield 300
                y_sb = y_pool.tile([P, QC], f32, tag="y", name="yq_sb")
                nc.vector.tensor_add(y_sb[:QT_TOK, :], yps[:QT_TOK, :],
                                     bias_sb[:QT_TOK, ocs])
                nc.sync.dma_start(out[row0:row0 + QT_TOK, ocs],
                                  y_sb[:QT_TOK, :])

        def emit_qproj_loads(q):
            ats = []
            for cc in range(n_cc):
                at = at_pool.tile([P, P], bf16, tag="at", name="atq_tile")
                nc.sync.dma_start(at[:, :QT_TOK],
                                  a2a_qout[q][cc * P:(cc + 1) * P, :])
                ats.append(at)
            return ats

        def emit_proj_loads(b, hf):
            ats = []
            for cc in range(n_cc):
                at = at_pool.tile([P, P], bf16, tag="at", name="at_tile")
                nc.sync.dma_start(at[:], a2a_out[b][hf][cc * P:(cc + 1) * P, :])
                ats.append(at)
            return ats

        # ---- main program ----
        # Batch loop is software-pipelined one stage deep: batch b-1's last
        # PV chunk and second AllToAll are carried into batch b's scores
        # region, so the PE keeps working through the exp of the next batch
        # and the ACT engine never waits on a batch boundary.
        xts = emit_xt(0, split=4)
        qt_sb = qkv_pool.tile([P, NQ], bf16, tag="qt")
        kt_sb = qkv_pool.tile([P, NQ], bf16, tag="kt")
        v_sb = qkv_pool.tile([P, n_kt, 2, 65], bf16, tag="v", bufs=3)
        # batch 0 bootstrap, ordered so the first scores pair can execute
        # as early as possible: KT token-chunk 0 and QT chunk 0 first (the
        # only inputs of the first score tiles), then the rest of KT; QT
        # chunks 1-3 and V interleave into the exp cadence
        finish(gen_w_chunk(xts, wk_sb, kt_sb, 0))
        g_qts = [gen_qt_qc(xts, qt_sb, qc) for qc in range(n_qc)]
        finish(g_qts[0])
        for qc in range(1, n_qc):
            finish(gen_w_chunk(xts, wk_sb, kt_sb, qc))
        g_kt = None
        g_v = gen_qkv_v(xts, v_sb)
        # all non-PV gens go through `slack` so the `work` tier is PV-only
        # (the singleton-slack interleave in pump() relies on that)
        slack.extend(g_qts[1:])
        slack.append(g_v)
        # W_proj + bias aren't needed until the first projection
        nc.sync.dma_start(wp_sb[:], wp.rearrange("(cc p) m -> p cc m", p=P))
        nc.sync.dma_start(bias_row[:], bp[:, :])
        nc.gpsimd.partition_broadcast(bias_sb[:], bias_row[:])

        g_pv_carry = g_pv_carry2 = None
        prev_ot = None
        for b in range(NB):
            # this batch's full KT must be emitted before its first scores
            # matmul (normally already drained by batch b-1's pumps)
            finish(g_kt)
            if b + 1 < NB:
                next_xts = emit_xt(b + 1)
                nqt = qkv_pool.tile([P, NQ], bf16, tag="qt", name="nqt")
                nkt = qkv_pool.tile([P, NQ], bf16, tag="kt", name="nkt")
                nv = qkv_pool.tile([P, n_kt, 2, 65], bf16, tag="v", name="nv", bufs=3)
            if b + 1 < NB:
                n_kt_g = gen_kt_all(next_xts, nkt)
                n_qts = [gen_qt_qc(next_xts, nqt, qc) for qc in range(n_qc)]
                n_v = gen_qkv_v(next_xts, nv)
                slack.append(n_kt_g)
                slack.append(n_v)
                slack.extend(n_qts)
            if b > 0:
                # proj is the least time-critical work (nothing reads its
                # output until the kernel end): keep it at the queue back as
                # the slack absorber
                ats0 = emit_proj_loads(b - 1, 0)
                slack.append(gen_proj(b - 1, 0, ats0))

            ot_sb = ot_pool.tile([P, NQ], bf16, tag="ot")
            pts = [None] * n_qc
            pv_gens = [None] * n_qc
            for qc in range(n_qc):
                # this batch's QT for qc must be fully emitted before the
                # scores that read it (normally already drained by pumps)
                finish(g_qts[qc])
                pts[qc] = emit_scores(qc, qt_sb, kt_sb)
                if qc == 0:
                    if DEBUG_DUMP and b == 0:
                        dpv = dbg_pt.rearrange("p (j q) -> p j q",
                                               j=2 * (NQ // P))
                        nc.sync.dma_start(dpv[:, 0:n_kt], pts[0][0][:])
                        nc.sync.dma_start(dpv[:, n_kt:2 * n_kt], pts[0][1][:])
                    if b > 0:
                        # batch b-1's tail: its last PVs ran during exp(b, 0)
                        finish(g_pv_carry2)
                        finish(g_pv_carry)
                        if DEBUG_DUMP and b == 1:
                            nc.sync.dma_start(dbg_ot[:, :], prev_ot[:])
                        emit_a2a(b - 1, 1, prev_ot)
                        ats1 = emit_proj_loads(b - 1, 1)
                        slack.append(gen_proj(b - 1, 1, ats1))
                # queue this q-chunk's PV at the FRONT of the work queue: it
                # is the dep-critical background work (its pt slots must
                # recycle for later exps) and fills the very next exp window
                finish(g_v)
                pv_gens[qc] = gen_pv(qc, pts[qc], v_sb, ot_sb,
                                     dbg=DEBUG_DUMP and b == 0 and qc == 0)
                work.appendleft(pv_gens[qc])
            finish(pv_gens[0])
            finish(pv_gens[1])
            emit_a2a(b, 0, ot_sb)
            if b == NB - 1:
                # the last batch's first-half projection can ride the exp
                # drain window instead of the serial tail
                ats_l0 = emit_proj_loads(b, 0)
                slack.append(gen_proj(b, 0, ats_l0))
            # force ~16us of backlog emission here: it executes during the
            # last q-chunk's exp drain and the batch boundary, where the PE
            # would otherwise idle (and keeps proj from piling up at the end)
            pump(16000)
            if DEBUG_DUMP and b == 0:
                nc.sync.dma_start(dbg_qt[:, :], qt_sb[:])
                nc.sync.dma_start(dbg_kt[:, :], kt_sb[:])
                nc.sync.dma_start(
                    dbg_v.rearrange("p (t g c) -> p t g c", t=NQ // P, g=2),
                    v_sb[:])
                nc.sync.dma_start(dbg_a2a[:, :], a2a_out[0][0][:])
            g_pv_carry2 = pv_gens[2]
            g_pv_carry = pv_gens[3]
            prev_ot = ot_sb
            if b + 1 < NB:
                qt_sb, kt_sb, v_sb = nqt, nkt, nv
                g_kt, g_qts, g_v = n_kt_g, n_qts, n_v
                xts = next_xts

        # final tail: qc2's quarter reshard + projection overlap exp(3,3)
        # and pv(3,3); only the qc3 quarter is serial
        finish(g_pv_carry2)
        emit_a2a_quarter(0, prev_ot)
        ats_q0 = emit_qproj_loads(0)
        slack.append(gen_proj_quarter(0, ats_q0))
        finish(g_pv_carry)
        emit_a2a_quarter(1, prev_ot)
        ats_q1 = emit_qproj_loads(1)
        slack.append(gen_proj_quarter(1, ats_q1))
        drain()

    nc.compile()
    return nc


def make_in_maps(x, W_qkv, W_proj, b_proj, NB=B, NQ=N, CH=C):
    """Shard the full inputs into one input map per core."""
    xT = np.ascontiguousarray(x.reshape(NB * NQ, CH).T).astype(BF16)
    wp = np.ascontiguousarray(W_proj).astype(BF16)
    bp = np.ascontiguousarray(b_proj[None, :]).astype(np.float32)
    in_maps = []
    for c in range(NCORES):
        cs = slice(P * c, P * (c + 1))
        in_maps.append({
            "xT": xT,
            "wq": np.ascontiguousarray(W_qkv[:, cs]).astype(BF16),
            "wk": np.ascontiguousarray(W_qkv[:, CH:][:, cs]).astype(BF16),
            "wv": np.ascontiguousarray(W_qkv[:, 2 * CH:][:, cs]).astype(BF16),
            "wp": wp,
            "bp": bp,
        })
    return in_maps


def assemble_output(results, NB=B, NQ=N, CH=C):
    """Concatenate the per-core token shards into the full output.

    Core c's out rows [b*256 + hf*128 .. +128] hold tokens
    [hf*1024 + 128c .. +128] of batch b."""
    TPB = NQ // NCORES
    QT = (NQ // 4) // NCORES  # 64: tokens per core per last-batch quarter
    full = np.empty((NB, NQ, CH), dtype=np.float32)
    for c in range(NCORES):
        y = np.asarray(results[c]["out"], dtype=np.float32)
        for b in range(NB):
            full[b, P * c:P * (c + 1), :] = y[b * TPB:b * TPB + P]
            if b < NB - 1:
                full[b, NQ // 2 + P * c:NQ // 2 + P * (c + 1), :] = \
                    y[b * TPB + P:b * TPB + 2 * P]
            else:
                # last batch's second half arrives as two quarter reshards
                for q in range(2):
                    tok0 = (2 + q) * (NQ // 4) + QT * c
                    r0 = b * TPB + P + q * QT
                    full[b, tok0:tok0 + QT, :] = y[r0:r0 + QT]
    return full


_compiled_nc = None


def kernel(x, W_qkv, W_proj, b_proj):
    global _compiled_nc
    x = np.asarray(x, dtype=np.float32)
    W_qkv = np.asarray(W_qkv, dtype=np.float32)
    W_proj = np.asarray(W_proj, dtype=np.float32)
    b_proj = np.asarray(b_proj, dtype=np.float32)

    if _compiled_nc is None:
        _compiled_nc = build_attention_nc()

    from concourse.bass_utils import run_bass_kernel_spmd

    in_maps = make_in_maps(x, W_qkv, W_proj, b_proj)
    res = run_bass_kernel_spmd(_compiled_nc, in_maps,
                               core_ids=list(range(NCORES)))
    return assemble_output(res.results)
